# revision 3
# baseline (speedup 1.0000x reference)
"""RLeaky SNN scan kernel for Trainium2 — flipped-matmul design, v4.

Same math as kernel2 (flip orientation validated at rel err 6.9e-8, 0 spike
flips vs the XLA reference), with tail pipelining:

- The per-step PE transposes take MEM (available one chain-op earlier than
  spk) and the threshold runs on the transposed PSUM directly, fusing the
  psum->sbuf copy with the is_gt: spkT_k = (mem^T > 1).
- spkT is 4 independent tiles, one per transpose, so the jb=0 matmul round
  of the next step only waits for transpose k=0 (+ its is_gt), while
  k=1..3 overlap with the first matmul rounds.
- b-major spike (for the reset subtract + DMA out) is computed in parallel
  on DVE.

Per step t (>0):
  u1   = 0.95 * mem          (ACT)    — overlaps the matmuls
  u2   = u1 + x_t            (DVE)    — overlaps the matmuls
  dot[32fc:+32, :] = sum_jb spkT(jb).T @ WT[jb, 512fc:+512]  (PSUM, jb asc,
                     4 column groups concurrent)
  u3   = u2 + dot            (DVE, reads PSUM)
  u4   = u3 + b              (DVE)
  mem' = u4 - spk            (DVE)
  spk' = (mem' > 1.0)        (DVE is_gt, b-major)
  tp_k = transpose(mem'[:, 128k:+128])   (PE x4)
  spkT'_k = (tp_k > 1.0)     (DVE is_gt, f-major stationary)
"""

import sys

if "/opt/trn_rl_repo" not in sys.path:
    sys.path.insert(0, "/opt/trn_rl_repo")

import numpy as np

import concourse.mybir as mybir
import concourse.tile as tile
from concourse import bacc
from concourse.bass_utils import run_bass_kernel_spmd

F32 = mybir.dt.float32

B, T_FULL, F = 256, 128, 2048
NCORES = 8
BL = B // NCORES  # 32 batch rows per core

_nc_cache = {}


def _build(T=T_FULL, repeat=1):
    key = (T, repeat)
    if key in _nc_cache:
        return _nc_cache[key]

    nc = bacc.Bacc(None, target_bir_lowering=False)
    xp_d = nc.dram_tensor("xp", [T, 128, 512], F32, kind="ExternalInput")
    wt_d = nc.dram_tensor("wt", [F, F], F32, kind="ExternalInput")  # W.T, [j, f]
    bp_d = nc.dram_tensor("bp", [128, 512], F32, kind="ExternalInput")
    eye_d = nc.dram_tensor("eye", [128, 128], F32, kind="ExternalInput")
    spk_out = nc.dram_tensor("spk_out", [T, 128, 512], F32, kind="ExternalOutput")
    mem_out = nc.dram_tensor("mem_out", [T, 128, 512], F32, kind="ExternalOutput")

    with tile.TileContext(nc) as tc:
        with (
            tc.tile_pool(name="wpool", bufs=1) as wpool,
            tc.tile_pool(name="wdma", bufs=2) as wdma,
            tc.tile_pool(name="const", bufs=1) as const,
            tc.tile_pool(name="state", bufs=2) as state,
            tc.tile_pool(name="xtp", bufs=3) as xtp,
            tc.tile_pool(name="tmp", bufs=2) as tmp,
            tc.tile_pool(name="pmm", bufs=2, space="PSUM") as pmm,
            tc.tile_pool(name="ptp", bufs=2, space="PSUM") as ptp,
        ):
            wt_sb = wpool.tile([128, 16 * F], F32)
            for jb in range(16):
                wchunk = wdma.tile([128, F], F32, tag="wchunk")
                nc.gpsimd.dma_start(wchunk[:], wt_d[jb * 128 : (jb + 1) * 128, :])
                nc.vector.tensor_copy(wt_sb[:, jb * F : (jb + 1) * F], wchunk[:])

            bp_sb = const.tile([128, 512], F32)
            nc.gpsimd.dma_start(bp_sb[:], bp_d[:])
            eye_sb = const.tile([128, 128], F32)
            nc.gpsimd.dma_start(eye_sb[:], eye_d[:])

            def make_spkT(mem_new, t, rep):
                # 4 PE transposes of mem slices -> psum; is_gt psum -> spkT_k
                tp_ps = ptp.tile([128, 512], F32, tag="tp", name=f"tp{rep}_{t}")
                spkT = []
                for k in range(4):
                    nc.tensor.transpose(
                        tp_ps[:, 128 * k : 128 * (k + 1)],
                        mem_new[:, 128 * k : 128 * (k + 1)],
                        eye_sb[:],
                    )
                    sT = state.tile(
                        [128, 128], F32, tag=f"spkT{k}", name=f"spkT{rep}_{t}_{k}"
                    )
                    nc.vector.tensor_scalar(
                        sT[:], tp_ps[:, 128 * k : 128 * (k + 1)],
                        1.0, None, mybir.AluOpType.is_gt,
                    )
                    spkT.append(sT)
                return spkT

            def scan_body(rep="r"):
                # t = 0: mem1 = x0 + b (bitwise equal to the reference chain
                # with mem0 = spk0 = 0), spk1 = mem1 > 1
                xt = xtp.tile([128, 512], F32, tag="xt", name=f"xt{rep}_0")
                nc.gpsimd.dma_start(xt[:], xp_d[0, :, :])
                mem_cur = state.tile([128, 512], F32, tag="mem", name=f"mem{rep}_1")
                nc.vector.tensor_add(mem_cur[:], xt[:], bp_sb[:])
                spk_cur = state.tile([128, 512], F32, tag="spk", name=f"spk{rep}_1")
                nc.vector.tensor_scalar(
                    spk_cur[:], mem_cur[:], 1.0, None, mybir.AluOpType.is_gt
                )
                nc.gpsimd.dma_start(mem_out[0, :, :], mem_cur[:])
                nc.gpsimd.dma_start(spk_out[0, :, :], spk_cur[:])
                spkT = make_spkT(mem_cur, 0, rep)

                for t in range(1, T):
                    xt = xtp.tile([128, 512], F32, tag="xt", name=f"xt{rep}_{t}")
                    nc.gpsimd.dma_start(xt[:], xp_d[t, :, :])

                    u1 = tmp.tile([128, 512], F32, tag="u1", name=f"u1_{t}")
                    nc.scalar.mul(u1[:], mem_cur[:], 0.95)
                    u2 = tmp.tile([128, 512], F32, tag="u2", name=f"u2_{t}")
                    nc.vector.tensor_add(u2[:], u1[:], xt[:])

                    # dot: 4 col-groups (f-chunks) x 16 jb, jb ascending
                    mm_ps = pmm.tile([128, 512], F32, tag="mm", name=f"mm{t}")
                    for jb in range(16):
                        st = spkT[jb % 4][:, 32 * (jb // 4) :][:, :32]
                        for fc in range(4):
                            nc.tensor.matmul(
                                mm_ps[32 * fc : 32 * (fc + 1), :],
                                st,
                                wt_sb[:, jb * F + 512 * fc : jb * F + 512 * (fc + 1)],
                                start=(jb == 0),
                                stop=(jb == 15),
                                tile_position=(0, 32 * fc),
                            )

                    u3 = tmp.tile([128, 512], F32, tag="u3", name=f"u3_{t}")
                    nc.vector.tensor_add(u3[:], u2[:], mm_ps[:])
                    u4 = tmp.tile([128, 512], F32, tag="u4", name=f"u4_{t}")
                    nc.vector.tensor_add(u4[:], u3[:], bp_sb[:])
                    mem_new = state.tile([128, 512], F32, tag="mem", name=f"mem{rep}_{t + 1}")
                    nc.vector.tensor_sub(mem_new[:], u4[:], spk_cur[:])

                    if t < T - 1:
                        spkT = make_spkT(mem_new, t, rep)

                    spk_new = state.tile([128, 512], F32, tag="spk", name=f"spk{rep}_{t + 1}")
                    nc.vector.tensor_scalar(
                        spk_new[:], mem_new[:], 1.0, None, mybir.AluOpType.is_gt
                    )
                    nc.gpsimd.dma_start(mem_out[t, :, :], mem_new[:])
                    nc.gpsimd.dma_start(spk_out[t, :, :], spk_new[:])

                    mem_cur = mem_new
                    spk_cur = spk_new

            if repeat == 1:
                scan_body()
            else:
                with tc.For_i(0, repeat, 1):
                    scan_body()

    nc.compile()
    _nc_cache[key] = nc
    return nc


def _pack_x(xc, T):
    # [32, T, 2048] -> [T, 128, 512] b-major packed:
    # out[t, 32*fc + i, f'] = xc[i, t, 512*fc + f']
    a = xc.reshape(BL, T, 4, 512).transpose(1, 2, 0, 3)  # [T, 4, 32, 512]
    return np.ascontiguousarray(a.reshape(T, 128, 512))


def _unpack_rec(a, T):
    # [T, 128, 512] b-major packed -> [32, T, 2048]
    a = a.reshape(T, 4, BL, 512).transpose(2, 0, 1, 3)  # [32, T, 4, 512]
    return np.ascontiguousarray(a.reshape(BL, T, 2048))


def _pack_b(b):
    return np.ascontiguousarray(
        np.repeat(b.reshape(4, 1, 512), BL, axis=1).reshape(128, 512)
    )


def kernel(x, W, b, T=None, trace=False, repeat=1):
    x = np.asarray(x, dtype=np.float32)
    W = np.asarray(W, dtype=np.float32)
    b = np.asarray(b, dtype=np.float32)
    if T is None:
        T = x.shape[1]
    x = x[:, :T, :]

    nc = _build(T, repeat=repeat)
    Wt = np.ascontiguousarray(W.T)
    bp = _pack_b(b)
    eye = np.eye(128, dtype=np.float32)

    in_maps = []
    for c in range(NCORES):
        xc = x[c * BL : (c + 1) * BL]  # [32, T, 2048]
        in_maps.append({"xp": _pack_x(xc, T), "wt": Wt, "bp": bp, "eye": eye})

    res = run_bass_kernel_spmd(
        nc, in_maps, core_ids=list(range(NCORES)), trace=trace
    )
    spk_parts = []
    mem_parts = []
    for c in range(NCORES):
        spk_parts.append(_unpack_rec(res.results[c]["spk_out"], T))
        mem_parts.append(_unpack_rec(res.results[c]["mem_out"], T))
    spk_rec = np.concatenate(spk_parts, axis=0)
    mem_rec = np.concatenate(mem_parts, axis=0)
    if trace:
        kernel.last_result = res
    return spk_rec, mem_rec
